# revision 9
# baseline (speedup 1.0000x reference)
"""CAPE connectivity loss on 8 Trainium2 NeuronCores.

Math (reference): fg_prob = softmax(logits, ch_axis)[:, 1] = sigmoid(l1 - l0);
per batch, heat diffuses from 32 source voxels for 10 iterations of
  h = avg_pool3d_3x3x3(h) * prob;  h /= (max(h) + 1e-5)
then scores = h[endpoints_b], loss = mean over batches of (1 - mean(scores)).

Key structure exploited:
 1. The 3x3x3 box filter dilates support by exactly 1 voxel per iteration, so
    after 10 iterations heat is identically zero outside L-inf radius-10 balls
    around the 32 sources. Compute only on per-cluster regions (bbox+10),
    merged until the expanded regions are pairwise disjoint - then zero-BC
    diffusion per region is exact.
 2. The per-iteration max-normalization commutes with the linear
    pool*prob step: iterate u_{k+1} = pool(u_k)*prob unnormalized, record
    mu_k = max(u_k); then h_k = u_k / c_k with c_k = mu_k + SMOOTH*c_{k-1}.
    Zero-BC values outside a piece's owned claim only *underestimate* the true
    field, and the argmax voxel lies inside some owned claim, so the max over
    all region boxes equals the true global max.
 3. Oversized regions are split along an axis with a +10-voxel halo per
    internal cut; each piece's owned claim stays exact for all 10 iterations.

Device layout: pieces are rotated (largest axis -> partitions), their source
bboxes centered, and packed into columns of [128, j, bh, bw] bf16 SBUF tiles
in two width classes, with zero guard bands around each box (2 cols left so
w-windows stay >= 2). prob = sigmoid(l1-l0) is computed on the host for the
packed boxes only and shipped directly. Per iteration on a band that tracks
the dilated support (even-aligned w-windows for DVE 2x mode), the separable
3x3x3 pool splits engine-adaptively: the D-axis 3-tap is a PE matmul against
a block-tridiagonal 1/27 matrix; the W-axis 3-tap always rides on PE as
w-shifted accumulating matmuls; the H-axis 3-tap runs as 0/1/2 DVE row-shift
adds with the remainder folded into extra h-shifted matmuls (3/6/9 matmuls
per chunk) - chosen per (class, iteration) by a cost model balancing DVE vs
PE load. The *prob multiply runs on DVE, reading PSUM directly for small
bands or via a ScalarE PSUM->SBUF copy (idle engine) for large ones. u is
updated in place; band slabs of u_8/u_9/u_10 are DMAed out and the host
computes the global maxima (scale recurrence c_10 = mu_10 + eps*mu_9 +
eps^2*mu_8 + O(eps^3)) and gathers endpoint_b values. A burst of dummy
matmuls at kernel start warms the PE HAM clock gate during the input DMA.
"""

import ml_dtypes
import numpy as np

B, C, D, H, W = 2, 2, 128, 256, 256
N_PAIRS = 32
N_ITERS = 10
SMOOTH = 1e-5
R = N_ITERS  # diffusion reach in voxels
VOL = (D, H, W)

AXIS_CAP = 34   # max free-axis extent of a piece (split with halo beyond)
P_CAP = 128     # max partition-axis extent
A_FREE = 21     # class-A pieces have both free extents <= A_FREE
N_CORES = 8
N_WARM = 64     # PE warmup matmuls (HAM un-throttle needs ~3.4us busy)
HOP_MIN = 256   # min band area for the ScalarE PSUM->SBUF hop

# Results of the last device run (for test harness introspection).
LAST_RESULTS = None


# --------------------------------------------------------------------------
# planning: clusters -> regions -> pieces
# --------------------------------------------------------------------------

class Piece:
    __slots__ = ("lo", "hi", "clo", "chi", "srcs", "batch",
                 "perm", "cls", "core", "col", "p0", "offh", "offw")

    def __init__(self, lo, hi, clo, chi, srcs, batch):
        self.lo = np.asarray(lo); self.hi = np.asarray(hi)
        self.clo = np.asarray(clo); self.chi = np.asarray(chi)
        self.srcs = srcs
        self.batch = batch

    @property
    def ext(self):
        return self.hi - self.lo + 1


def _merge_clusters(pts):
    """Merge clusters until expanded regions (bbox+R) are pairwise disjoint."""
    clusters = [[i] for i in range(len(pts))]

    def bbox(c):
        p = pts[c]
        return p.min(0), p.max(0)

    changed = True
    while changed:
        changed = False
        out = []
        while clusters:
            c = clusters.pop()
            lo_c, hi_c = bbox(c)
            for k, o in enumerate(clusters):
                lo_o, hi_o = bbox(o)
                if np.all(lo_c - hi_o <= 2 * R) and np.all(lo_o - hi_c <= 2 * R):
                    clusters[k] = o + c
                    changed = True
                    break
            else:
                out.append(c)
        clusters = out
    return clusters


def _split(piece):
    """Split a piece until partition extent <= P_CAP and free extents <=
    AXIS_CAP. Claims are halved; each internal cut adds R halo."""
    out, done = [piece], []
    while out:
        p = out.pop()
        ext = p.ext
        order = np.argsort(-ext, kind="stable")
        ax = None
        if ext[order[0]] > P_CAP:
            ax = order[0]
        elif ext[order[1]] > AXIS_CAP:
            ax = order[1]
        elif ext[order[2]] > AXIS_CAP:
            ax = order[2]
        if ax is None:
            done.append(p)
            continue
        mid = (p.clo[ax] + p.chi[ax]) // 2
        for a, b in ((p.clo[ax], mid), (mid + 1, p.chi[ax])):
            nclo, nchi = p.clo.copy(), p.chi.copy()
            nclo[ax], nchi[ax] = a, b
            nlo, nhi = p.lo.copy(), p.hi.copy()
            nlo[ax] = max(a - R, p.lo[ax])
            nhi[ax] = min(b + R, p.hi[ax])
            srcs = [s for s in p.srcs if nlo[ax] <= s[ax] <= nhi[ax]]
            out.append(Piece(nlo, nhi, nclo, nchi, srcs, p.batch))
    return done


def make_pieces(ea):
    pieces = []
    for b in range(ea.shape[0]):
        pts = np.unique(ea[b], axis=0)
        for cl in _merge_clusters(pts):
            p = pts[cl]
            lo = np.maximum(p.min(0) - R, 0)
            hi = np.minimum(p.max(0) + R, np.asarray(VOL) - 1)
            base = Piece(lo, hi, lo, hi, [tuple(x) for x in p], b)
            pieces.extend(_split(base))
    return pieces


# --------------------------------------------------------------------------
# packing: pieces -> (class, core, col, partition offset) + schedule
# --------------------------------------------------------------------------

class Cfg:
    pass


def _chunks(rows, wn):
    nch = max(1, -(-(rows * wn) // 512))
    rpc = -(-rows // nch)
    while rpc * wn > 512:
        nch += 1
        rpc = -(-rows // nch)
    return nch, rpc


def pack(pieces):
    """Assign each piece a rotation + (class, core, col, p0); build the
    per-(class, iteration) band slabs, chunking, and engine schedule."""
    for p in pieces:
        ext = p.ext
        p.perm = tuple(int(i) for i in np.argsort(-ext, kind="stable"))

    def free_ext(p):
        e = p.ext
        return e[p.perm[1]], e[p.perm[2]]

    cls_of = {}
    for p in pieces:
        fh, fw = free_ext(p)
        p.cls = 0 if (fh <= A_FREE and fw <= A_FREE) else 1
        cls_of.setdefault(p.cls, []).append(p)

    cfg = Cfg()
    cfg.classes = []
    for cls in (0, 1):
        plist = cls_of.get(cls, [])
        if not plist:
            continue
        ih = max(int(free_ext(p)[0]) for p in plist)
        iw = max(int(free_ext(p)[1]) for p in plist)
        bh = ih + 2
        bw = iw + 4 + ((iw + 4) % 2)  # 2 guard cols left, >=2 right, even
        # first-fit decreasing bin pack by partition extent
        plist.sort(key=lambda p: -int(p.ext[p.perm[0]]))
        bins = []  # (used, [pieces])
        for p in plist:
            pe = int(p.ext[p.perm[0]])
            for b_ in bins:
                if b_[0] + pe <= 128:
                    p.p0 = b_[0]
                    b_[0] += pe
                    b_[1].append(p)
                    break
            else:
                p.p0 = 0
                bins.append([pe, [p]])
        cfg.classes.append(dict(cls=cls, ih=ih, iw=iw, bh=bh, bw=bw,
                                cols=bins))

    for c in cfg.classes:
        for i, b_ in enumerate(c["cols"]):
            for p in b_[1]:
                p.core = i % N_CORES
                p.col = i // N_CORES
        c["j"] = (len(c["cols"]) + N_CORES - 1) // N_CORES

    # Center each piece's source bbox in its box interior (rows start at 1,
    # cols at 2 so shifted matmul reads never cross the box start), then
    # per-class per-iteration band slabs (h0,h1,w0,w1) with the w-window
    # even-aligned for DVE 2x mode.
    for c in cfg.classes:
        ih, iw = c["ih"], c["iw"]
        ulo = [10**9, 10**9]
        uhi = [-1, -1]
        for b_ in c["cols"]:
            for p in b_[1]:
                offs = []
                for axi, ilen, base in ((1, ih, 1), (2, iw, 2)):
                    ax = p.perm[axi]
                    ext = int(p.ext[ax])
                    if p.srcs:
                        slo = min(s[ax] for s in p.srcs) - int(p.lo[ax])
                        shi = max(s[ax] for s in p.srcs) - int(p.lo[ax])
                    else:
                        slo = shi = ext // 2
                    start = base + (ilen - 1) // 2 - (slo + shi) // 2
                    start = min(max(start, base), base + ilen - ext)
                    offs.append(start)
                    if p.srcs:
                        i01 = 0 if axi == 1 else 1
                        ulo[i01] = min(ulo[i01], start + slo)
                        uhi[i01] = max(uhi[i01], start + shi)
                p.offh, p.offw = offs
        if uhi[0] < 0:  # no sources in this class at all
            ulo, uhi = [1, 2], [ih, iw + 1]
        c["ext_k"] = []
        for k in range(1, N_ITERS + 1):
            h0 = max(1, ulo[0] - k)
            h1 = min(1 + ih, uhi[0] + k + 1)
            w0 = max(2, ulo[1] - k)
            w1 = min(2 + iw, uhi[1] + k + 1)
            w0 -= w0 % 2
            wn = w1 - w0
            wn += wn % 2
            c["ext_k"].append((int(h0), int(h1), int(w0), int(w0 + wn)))

    # Engine schedule. The PE is power-throttled to ~50% duty under
    # sustained load (~1.2 ns/col effective), so DVE 2x (0.53 ns/elem) is
    # the fastest engine per element: the W-axis 3-tap always runs as 2 DVE
    # shift-adds; per (iteration, class) the H-axis 3-tap runs as n_t in
    # {2, 1, 0} further DVE ops (2 = full H on DVE + 1 matmul; 1 = H pair
    # on DVE + 2 matmuls; 0 = H folded into 3 h-shifted matmuls), greedily
    # balancing cumulative DVE vs PE busy-time. The psum->u multiply hops
    # through a chunk-pipelined ScalarE copy (idle engine) for bands >=
    # HOP_MIN so the DVE multiply runs from SBUF at 2x.
    dve = pe = 0.0
    for it in range(N_ITERS):
        for c in cfg.classes:
            j = c["j"]
            h0, h1, w0, w1 = c["ext_k"][it]
            rows, wn = h1 - h0, w1 - w0
            A = rows * wn * j
            nch, rpc = _chunks(rows, wn)
            ach = rpc * wn
            hop = A >= HOP_MIN
            mcost = (nch * (170 + 0.53 * ach) if hop
                     else nch * (192 + 1.25 * ach))
            best = None
            for n in (2, 1, 0):
                d = (2 + n) * (170 + 0.53 * A) + mcost * j
                pcost = nch * (3 - n) * (ach * 0.9 + 160) * j
                tot = max(dve + d, pe + pcost)
                if best is None or tot < best[0]:
                    best = (tot, n, d, pcost)
            _, n, d, pcost = best
            dve += d
            pe += pcost
            c.setdefault("sched", []).append(
                dict(n_t=n, hop=hop, nch=nch, rpc=rpc))
    cfg.pieces = pieces
    return cfg


# --------------------------------------------------------------------------
# host-side data packing
# --------------------------------------------------------------------------

def build_inputs(cfg, logits, ea):
    """Build per-core input arrays. Returns in_maps (list of dicts)."""
    bf16 = ml_dtypes.bfloat16
    in_maps = [dict() for _ in range(N_CORES)]
    for ci, c in enumerate(cfg.classes):
        j, bh, bw = c["j"], c["bh"], c["bw"]
        for core in range(N_CORES):
            # prob defaults to 0 outside regions: no heat leaks through box
            # cells beyond a (clipped) region slab, matching the true zero BC.
            in_maps[core][f"pr_{ci}"] = np.zeros((128, j, bh, bw), bf16)
            in_maps[core][f"u0_{ci}"] = np.zeros((128, j, bh, bw), bf16)
            in_maps[core][f"tm_{ci}"] = np.zeros((128, j, 128), bf16)

    inv27 = np.float32(1.0 / 27.0)
    for p in cfg.pieces:
        ci = next(i for i, c in enumerate(cfg.classes) if c["cls"] == p.cls)
        lo, hi, perm = p.lo, p.hi, p.perm
        pe = int(p.ext[perm[0]])
        eh = int(p.ext[perm[1]])
        ew = int(p.ext[perm[2]])
        sl = tuple(slice(int(lo[a]), int(hi[a]) + 1) for a in range(3))
        oh, ow = p.offh, p.offw
        dlt = (logits[p.batch, 1][sl].astype(np.float32)
               - logits[p.batch, 0][sl].astype(np.float32)).transpose(perm)
        prob = 1.0 / (1.0 + np.exp(-dlt))
        in_maps[p.core][f"pr_{ci}"][p.p0:p.p0 + pe, p.col,
                                    oh:oh + eh, ow:ow + ew] = \
            prob.astype(bf16)
        u0 = in_maps[p.core][f"u0_{ci}"]
        for s in p.srcs:
            q = (s[perm[0]] - lo[perm[0]], s[perm[1]] - lo[perm[1]],
                 s[perm[2]] - lo[perm[2]])
            u0[p.p0 + q[0], p.col, oh + q[1], ow + q[2]] = 1.0
        tm = in_maps[p.core][f"tm_{ci}"]
        for i in range(pe):
            for d_ in (-1, 0, 1):
                if 0 <= i + d_ < pe:
                    tm[p.p0 + i, p.col, p.p0 + i + d_] = inv27
    return in_maps


# --------------------------------------------------------------------------
# device kernel
# --------------------------------------------------------------------------

def build_nc(cfg):
    import concourse.bacc as bacc
    import concourse.tile as tile
    from concourse import mybir

    nc = bacc.Bacc("TRN2")
    dram = {}
    for ci, c in enumerate(cfg.classes):
        j, bh, bw = c["j"], c["bh"], c["bw"]
        for nm in ("pr", "u0"):
            dram[f"{nm}_{ci}"] = nc.dram_tensor(
                f"{nm}_{ci}", [128, j, bh, bw], mybir.dt.bfloat16,
                kind="ExternalInput")
        dram[f"tm_{ci}"] = nc.dram_tensor(
            f"tm_{ci}", [128, j, 128], mybir.dt.bfloat16, kind="ExternalInput")
        # full-width row ranges: per-partition contiguous runs so the DMA
        # moves ~2KB packets instead of one tiny packet per row
        for oi in (8, 9, 10):
            h0, h1, w0, w1 = c["ext_k"][oi - 1]
            dram[f"o{oi}_{ci}"] = nc.dram_tensor(
                f"o{oi}_{ci}", [128, j, h1 - h0, bw],
                mybir.dt.bfloat16, kind="ExternalOutput")

    with tile.TileContext(nc) as tc:
        with tc.tile_pool(name="sb", bufs=1) as sb, \
             tc.tile_pool(name="ps", bufs=7, space="PSUM") as pp:
            tiles = []
            for ci, c in enumerate(cfg.classes):
                j, bh, bw = c["j"], c["bh"], c["bw"]
                u = sb.tile([128, j, bh, bw], mybir.dt.bfloat16, tag=f"u{ci}")
                t1 = sb.tile([128, j, bh, bw], mybir.dt.bfloat16,
                             tag=f"t1{ci}")
                t2 = sb.tile([128, j, bh, bw], mybir.dt.bfloat16,
                             tag=f"t2{ci}")
                pr = sb.tile([128, j, bh, bw], mybir.dt.bfloat16,
                             tag=f"pr{ci}")
                tm = sb.tile([128, j, 128], mybir.dt.bfloat16, tag=f"tm{ci}")
                tiles.append((u, t1, t2, pr, tm))
            # inputs spread over three trigger queues (parallel DMA
            # engines); u0_0 gates the first taps, so its transfer is
            # split across two queues to halve the per-packet serial cost
            u0t = tiles[0][0]
            nc.sync.dma_start(out=u0t[0:64], in_=dram["u0_0"][0:64])
            nc.gpsimd.dma_start(out=u0t[64:128], in_=dram["u0_0"][64:128])
            for ci, c in enumerate(cfg.classes):
                u, t1, t2, pr, tm = tiles[ci]
                qa = nc.sync if ci == 0 else nc.gpsimd
                if ci > 0:
                    qa.dma_start(out=u[:], in_=dram[f"u0_{ci}"][:])
                qa.dma_start(out=tm[:], in_=dram[f"tm_{ci}"][:])
                nc.scalar.dma_start(out=pr[:], in_=dram[f"pr_{ci}"][:])

            def emit_taps(ci, it):
                # W-axis 3-tap on DVE (2 shift-adds), written 1 row wider
                # than the band so later H-tap/matmul reads touch only
                # written cells (u's ring is zero from the u0 DMA - no
                # memsets needed). Then the H-axis 3-tap as n_t in {2,1,0}
                # more DVE ops (the rest rides on PE as shifted matmuls).
                c = cfg.classes[ci]
                u, t1, t2, pr, tm = tiles[ci]
                h0, h1, w0, w1 = c["ext_k"][it]
                n_t = c["sched"][it]["n_t"]
                g0, g1 = h0 - 1, h1 + 1
                nc.vector.tensor_add(t1[:, :, g0:g1, w0:w1],
                                     u[:, :, g0:g1, w0 - 1:w1 - 1],
                                     u[:, :, g0:g1, w0 + 1:w1 + 1])
                nc.vector.tensor_add(t1[:, :, g0:g1, w0:w1],
                                     t1[:, :, g0:g1, w0:w1],
                                     u[:, :, g0:g1, w0:w1])
                if n_t >= 1:
                    nc.vector.tensor_add(t2[:, :, h0:h1, w0:w1],
                                         t1[:, :, h0 - 1:h1 - 1, w0:w1],
                                         t1[:, :, h0 + 1:h1 + 1, w0:w1])
                if n_t == 2:
                    nc.vector.tensor_add(t2[:, :, h0:h1, w0:w1],
                                         t2[:, :, h0:h1, w0:w1],
                                         t1[:, :, h0:h1, w0:w1])

            def emit_mms(ci, it):
                # D-axis tridiagonal matmul per chunk; residual H-axis taps
                # ride along as h-shifted accumulating matmuls. Then each
                # psum chunk hops through ScalarE (PSUM->SBUF bf16, idle
                # engine, overwriting t2) for bands >= HOP_MIN.
                c = cfg.classes[ci]
                u, t1, t2, pr, tm = tiles[ci]
                h0, h1, w0, w1 = c["ext_k"][it]
                sch = c["sched"][it]
                n_t, nch, rpc = sch["n_t"], sch["nch"], sch["rpc"]
                if n_t == 2:       # t2 = full H 3-tap sum
                    srcs = [(t2, 0)]
                elif n_t == 1:     # center (t1) first: it depends only on
                    # the W taps, so it overlaps H1 on DVE; t2 accumulates
                    srcs = [(t1, 0), (t2, 0)]
                else:              # full H 3-tap as shifted matmuls
                    srcs = [(t1, 0), (t1, -1), (t1, 1)]
                psums = []
                for jj in range(c["j"]):
                    for ch in range(nch):
                        r0 = h0 + ch * rpc
                        nr = min(rpc, h1 - r0)
                        ps = pp.tile([128, nr, w1 - w0],
                                     mybir.dt.float32, tag="ps")
                        for mi, (src, dh) in enumerate(srcs):
                            nc.tensor.matmul(
                                ps[:],
                                tm[:, jj, :],
                                src[:, jj, r0 + dh:r0 + dh + nr, w0:w1],
                                start=(mi == 0),
                                stop=(mi == len(srcs) - 1))
                        psums.append((jj, r0, nr, ps))
                if sch["hop"]:
                    # chunk 0 skips the hop (its multiply reads PSUM
                    # directly) - it sits first on the critical path and
                    # saving the ScalarE round-trip starts it ~0.6us sooner
                    for ki, (jj, r0, nr, ps) in enumerate(psums):
                        if ki == 0 and len(psums) >= 2:
                            continue
                        nc.scalar.activation(
                            t2[:, jj, r0:r0 + nr, w0:w1], ps[:],
                            mybir.ActivationFunctionType.Copy)
                return psums

            def emit_mults(ci, it, psums):
                # u = psum * prob (1/27 folded into tm), chunk-level so
                # early chunks run while later chunks are still matmuling.
                c = cfg.classes[ci]
                u, t1, t2, pr, tm = tiles[ci]
                h0, h1, w0, w1 = c["ext_k"][it]
                hop = c["sched"][it]["hop"]
                for ki, (jj, r0, nr, ps) in enumerate(psums):
                    direct = (not hop) or (ki == 0 and len(psums) >= 2)
                    src = ps[:] if direct else t2[:, jj, r0:r0 + nr, w0:w1]
                    nc.vector.tensor_mul(u[:, jj, r0:r0 + nr, w0:w1],
                                         src, pr[:, jj, r0:r0 + nr, w0:w1])
                if it >= 7:
                    if it == N_ITERS - 1 and c["j"] == 1 and len(psums) > 1:
                        # final iteration: per-chunk slabs so the last
                        # transfer starts as soon as its mult lands instead
                        # of waiting for the whole band
                        for ki, (jj, r0, nr, ps) in enumerate(psums):
                            q = (nc.gpsimd, nc.sync)[ki % 2]
                            q.dma_start(
                                out=dram[f"o{it + 1}_{ci}"]
                                    [:, :, r0 - h0:r0 - h0 + nr, :],
                                in_=u[:, :, r0:r0 + nr, :])
                    else:
                        # gpsimd queue: idle mid-kernel, so the slab
                        # transfer starts immediately and the WAR on u
                        # clears sooner
                        nc.gpsimd.dma_start(out=dram[f"o{it + 1}_{ci}"][:],
                                            in_=u[:, :, h0:h1, :])

            # Software-pipelined schedule: class 1's psum->u multiplies are
            # deferred into the next iteration so the in-order DVE queue
            # always has ready work (class 0's taps/multiply) while class
            # 1's matmul+copy chain drains. The Tile scheduler would undo
            # this (its cost model assumes an unthrottled PE and hoists the
            # stalling multiplies), so each phase is pinned with a
            # monotonically increasing bass_wait_until_ts pseudo-time.
            def tw(ns):
                tc.tile_set_cur_wait(ns * 1e-6)

            def dur(ci, it, what):
                c = cfg.classes[ci]
                h0, h1, w0, w1 = c["ext_k"][it]
                A = (h1 - h0) * (w1 - w0) * c["j"]
                sch = c["sched"][it]
                if what == "taps":
                    return (2 + sch["n_t"]) * (190 + 0.55 * A)
                if what == "mult":
                    per = (170 + 0.53 * A / sch["nch"]) if sch["hop"] \
                        else (192 + 1.25 * A / sch["nch"])
                    return sch["nch"] * per
                ach = A / sch["nch"]
                return sch["nch"] * (3 - sch["n_t"]) * (ach * 0.9 + 160)

            if len(cfg.classes) == 1:
                for it in range(N_ITERS):
                    emit_taps(0, it)
                    emit_mults(0, it, emit_mms(0, it))
            else:
                pend = None
                t = 3000.0
                for it in range(N_ITERS - 1):
                    tw(t)
                    emit_taps(0, it)
                    tb = t + dur(0, it, "taps")
                    tw(tb)
                    ps0 = emit_mms(0, it)
                    if pend is not None:
                        tw(tb + 200)
                        emit_mults(1, it - 1, pend)
                        tb += 200 + dur(1, it - 1, "mult")
                    tw(tb + 100)
                    emit_mults(0, it, ps0)
                    tb += 100 + dur(0, it, "mult")
                    tw(tb)
                    emit_taps(1, it)
                    tb += dur(1, it, "taps")
                    tw(tb)
                    pend = emit_mms(1, it)
                    t = tb + 400
                # Final iteration with roles swapped: class 1 (the long
                # matmul+copy chain) is issued first and class 0's short
                # chain drains the tail.
                it = N_ITERS - 1
                tw(t)
                emit_mults(1, it - 1, pend)
                t += dur(1, it - 1, "mult")
                tw(t)
                emit_taps(1, it)
                t += dur(1, it, "taps")
                tw(t)
                ps1 = emit_mms(1, it)
                tw(t + 200)
                emit_taps(0, it)
                t += 200 + dur(0, it, "taps")
                tw(t)
                ps0 = emit_mms(0, it)
                tw(t + 1400)
                emit_mults(1, it, ps1)
                tw(t + 1400 + dur(1, it, "mult"))
                emit_mults(0, it, ps0)
    nc.compile()
    return nc


# --------------------------------------------------------------------------
# host-side finalization
# --------------------------------------------------------------------------

def finalize(cfg, results, eb):
    """results: list of per-core dicts with o8/o9/o10 band slabs."""
    cls_idx = {c["cls"]: i for i, c in enumerate(cfg.classes)}

    # Global maxima of u_8/u_9/u_10 from the slabs; c_10 = mu_10 + eps*mu_9
    # + eps^2*mu_8 + O(eps^3) with eps = SMOOTH = 1e-5 (~1e-15 truncation).
    mus = np.zeros((B, 3), dtype=np.float64)
    for p in cfg.pieces:
        ci = cls_idx[p.cls]
        pe = int(p.ext[p.perm[0]])
        for oi in (8, 9, 10):
            m = results[p.core][f"o{oi}_{ci}"][p.p0:p.p0 + pe, p.col]
            mus[p.batch, oi - 8] = max(mus[p.batch, oi - 8],
                                       float(m.max()))

    per_batch = []
    for b in range(B):
        cscale = 1.0
        for it in range(3):
            if mus[b, it] > 0:
                cscale = mus[b, it] + SMOOTH * cscale
        scores = []
        for e in eb[b]:
            val = 0.0
            for p in cfg.pieces:
                if p.batch != b:
                    continue
                if np.all(p.clo <= e) and np.all(e <= p.chi):
                    ci = cls_idx[p.cls]
                    c = cfg.classes[ci]
                    h0, h1, _, _ = c["ext_k"][N_ITERS - 1]
                    q = (int(e[p.perm[0]] - p.lo[p.perm[0]]),
                         int(e[p.perm[1]] - p.lo[p.perm[1]]),
                         int(e[p.perm[2]] - p.lo[p.perm[2]]))
                    hs = p.offh + q[1] - h0
                    ws = p.offw + q[2]
                    if 0 <= hs < h1 - h0:
                        val = float(results[p.core][f"o10_{ci}"]
                                    [p.p0 + q[0], p.col, hs, ws])
                    break
            scores.append(val / cscale)
        per_batch.append(1.0 - np.float32(np.mean(np.asarray(scores,
                                                             np.float32))))
    return np.array(np.mean(np.asarray(per_batch, np.float32)),
                    dtype=np.float32)


# --------------------------------------------------------------------------
# entry point
# --------------------------------------------------------------------------

def kernel(logits, labels, endpoints_a, endpoints_b):
    global LAST_RESULTS
    logits = np.asarray(logits)
    ea = np.asarray(endpoints_a).astype(np.int64)
    eb = np.asarray(endpoints_b).astype(np.int64)

    cfg = pack(make_pieces(ea))
    in_maps = build_inputs(cfg, logits, ea)
    nc = build_nc(cfg)

    from concourse.bass_utils import run_bass_kernel_spmd
    res = run_bass_kernel_spmd(nc, in_maps, core_ids=list(range(N_CORES)))
    LAST_RESULTS = res
    return finalize(cfg, res.results, eb)


if __name__ == "__main__":
    ins = {k: np.load(f"/tmp/in_{k}.npy")
           for k in ("logits", "labels", "endpoints_a", "endpoints_b")}
    out = kernel(**ins)
    print("kernel loss:", repr(out))



# revision 11
# speedup vs baseline: 1.0178x; 1.0178x over previous
"""CAPE connectivity loss on 8 Trainium2 NeuronCores.

Math (reference): fg_prob = softmax(logits, ch_axis)[:, 1] = sigmoid(l1 - l0);
per batch, heat diffuses from 32 source voxels for 10 iterations of
  h = avg_pool3d_3x3x3(h) * prob;  h /= (max(h) + 1e-5)
then scores = h[endpoints_b], loss = mean over batches of (1 - mean(scores)).

Key structure exploited:
 1. The 3x3x3 box filter dilates support by exactly 1 voxel per iteration, so
    after 10 iterations heat is identically zero outside L-inf radius-10 balls
    around the 32 sources. Compute only on per-cluster regions (bbox+10),
    merged until the expanded regions are pairwise disjoint - then zero-BC
    diffusion per region is exact.
 2. The per-iteration max-normalization commutes with the linear
    pool*prob step: iterate u_{k+1} = pool(u_k)*prob unnormalized, record
    mu_k = max(u_k); then h_k = u_k / c_k with c_k = mu_k + SMOOTH*c_{k-1}.
    Zero-BC values outside a piece's owned claim only *underestimate* the true
    field, and the argmax voxel lies inside some owned claim, so the max over
    all region boxes equals the true global max.
 3. Oversized regions are split along an axis with a +10-voxel halo per
    internal cut; each piece's owned claim stays exact for all 10 iterations.

Device layout: pieces are rotated (largest axis -> partitions), their source
bboxes centered, and packed into columns of [128, j, bh, bw] bf16 SBUF tiles
in two width classes, with zero guard bands around each box (2 cols left so
w-windows stay >= 2). prob = sigmoid(l1-l0) is computed on the host for the
packed boxes only and shipped directly. Per iteration on a band that tracks
the dilated support (even-aligned w-windows for DVE 2x mode), the separable
3x3x3 pool splits engine-adaptively: the D-axis 3-tap is a PE matmul against
a block-tridiagonal 1/27 matrix; the W-axis 3-tap always rides on PE as
w-shifted accumulating matmuls; the H-axis 3-tap runs as 0/1/2 DVE row-shift
adds with the remainder folded into extra h-shifted matmuls (3/6/9 matmuls
per chunk) - chosen per (class, iteration) by a cost model balancing DVE vs
PE load. The *prob multiply runs on DVE, reading PSUM directly for small
bands or via a ScalarE PSUM->SBUF copy (idle engine) for large ones. u is
updated in place; band slabs of u_8/u_9/u_10 are DMAed out and the host
computes the global maxima (scale recurrence c_10 = mu_10 + eps*mu_9 +
eps^2*mu_8 + O(eps^3)) and gathers endpoint_b values. A burst of dummy
matmuls at kernel start warms the PE HAM clock gate during the input DMA.
"""

import ml_dtypes
import numpy as np

B, C, D, H, W = 2, 2, 128, 256, 256
N_PAIRS = 32
N_ITERS = 10
SMOOTH = 1e-5
R = N_ITERS  # diffusion reach in voxels
VOL = (D, H, W)

AXIS_CAP = 34   # max free-axis extent of a piece (split with halo beyond)
P_CAP = 128     # max partition-axis extent
A_FREE = 21     # class-A pieces have both free extents <= A_FREE
N_CORES = 8
N_WARM = 64     # PE warmup matmuls (HAM un-throttle needs ~3.4us busy)
HOP_MIN = 256   # min band area for the ScalarE PSUM->SBUF hop

# Results of the last device run (for test harness introspection).
LAST_RESULTS = None


# --------------------------------------------------------------------------
# planning: clusters -> regions -> pieces
# --------------------------------------------------------------------------

class Piece:
    __slots__ = ("lo", "hi", "clo", "chi", "srcs", "batch",
                 "perm", "cls", "core", "col", "p0", "offh", "offw")

    def __init__(self, lo, hi, clo, chi, srcs, batch):
        self.lo = np.asarray(lo); self.hi = np.asarray(hi)
        self.clo = np.asarray(clo); self.chi = np.asarray(chi)
        self.srcs = srcs
        self.batch = batch

    @property
    def ext(self):
        return self.hi - self.lo + 1


def _merge_clusters(pts):
    """Merge clusters until expanded regions (bbox+R) are pairwise disjoint."""
    clusters = [[i] for i in range(len(pts))]

    def bbox(c):
        p = pts[c]
        return p.min(0), p.max(0)

    changed = True
    while changed:
        changed = False
        out = []
        while clusters:
            c = clusters.pop()
            lo_c, hi_c = bbox(c)
            for k, o in enumerate(clusters):
                lo_o, hi_o = bbox(o)
                if np.all(lo_c - hi_o <= 2 * R) and np.all(lo_o - hi_c <= 2 * R):
                    clusters[k] = o + c
                    changed = True
                    break
            else:
                out.append(c)
        clusters = out
    return clusters


def _split(piece):
    """Split a piece until partition extent <= P_CAP and free extents <=
    AXIS_CAP. Claims are halved; each internal cut adds R halo."""
    out, done = [piece], []
    while out:
        p = out.pop()
        ext = p.ext
        order = np.argsort(-ext, kind="stable")
        ax = None
        if ext[order[0]] > P_CAP:
            ax = order[0]
        elif ext[order[1]] > AXIS_CAP:
            ax = order[1]
        elif ext[order[2]] > AXIS_CAP:
            ax = order[2]
        if ax is None:
            done.append(p)
            continue
        mid = (p.clo[ax] + p.chi[ax]) // 2
        for a, b in ((p.clo[ax], mid), (mid + 1, p.chi[ax])):
            nclo, nchi = p.clo.copy(), p.chi.copy()
            nclo[ax], nchi[ax] = a, b
            nlo, nhi = p.lo.copy(), p.hi.copy()
            nlo[ax] = max(a - R, p.lo[ax])
            nhi[ax] = min(b + R, p.hi[ax])
            srcs = [s for s in p.srcs if nlo[ax] <= s[ax] <= nhi[ax]]
            out.append(Piece(nlo, nhi, nclo, nchi, srcs, p.batch))
    return done


def make_pieces(ea):
    pieces = []
    for b in range(ea.shape[0]):
        pts = np.unique(ea[b], axis=0)
        for cl in _merge_clusters(pts):
            p = pts[cl]
            lo = np.maximum(p.min(0) - R, 0)
            hi = np.minimum(p.max(0) + R, np.asarray(VOL) - 1)
            base = Piece(lo, hi, lo, hi, [tuple(x) for x in p], b)
            pieces.extend(_split(base))
    return pieces


# --------------------------------------------------------------------------
# packing: pieces -> (class, core, col, partition offset) + schedule
# --------------------------------------------------------------------------

class Cfg:
    pass


def _chunks(rows, wn):
    nch = max(1, -(-(rows * wn) // 512))
    rpc = -(-rows // nch)
    while rpc * wn > 512:
        nch += 1
        rpc = -(-rows // nch)
    return nch, rpc


def pack(pieces):
    """Assign each piece a rotation + (class, core, col, p0); build the
    per-(class, iteration) band slabs, chunking, and engine schedule."""
    for p in pieces:
        ext = p.ext
        p.perm = tuple(int(i) for i in np.argsort(-ext, kind="stable"))

    def free_ext(p):
        e = p.ext
        return e[p.perm[1]], e[p.perm[2]]

    cls_of = {}
    for p in pieces:
        fh, fw = free_ext(p)
        p.cls = 0 if (fh <= A_FREE and fw <= A_FREE) else 1
        cls_of.setdefault(p.cls, []).append(p)

    cfg = Cfg()
    cfg.classes = []
    for cls in (0, 1):
        plist = cls_of.get(cls, [])
        if not plist:
            continue
        ih = max(int(free_ext(p)[0]) for p in plist)
        iw = max(int(free_ext(p)[1]) for p in plist)
        bh = ih + 2
        bw = iw + 4 + ((iw + 4) % 2)  # 2 guard cols left, >=2 right, even
        # first-fit decreasing bin pack by partition extent
        plist.sort(key=lambda p: -int(p.ext[p.perm[0]]))
        bins = []  # (used, [pieces])
        for p in plist:
            pe = int(p.ext[p.perm[0]])
            for b_ in bins:
                if b_[0] + pe <= 128:
                    p.p0 = b_[0]
                    b_[0] += pe
                    b_[1].append(p)
                    break
            else:
                p.p0 = 0
                bins.append([pe, [p]])
        cfg.classes.append(dict(cls=cls, ih=ih, iw=iw, bh=bh, bw=bw,
                                cols=bins))

    for c in cfg.classes:
        for i, b_ in enumerate(c["cols"]):
            for p in b_[1]:
                p.core = i % N_CORES
                p.col = i // N_CORES
        c["j"] = (len(c["cols"]) + N_CORES - 1) // N_CORES

    # Center each piece's source bbox in its box interior (rows start at 1,
    # cols at 2 so shifted matmul reads never cross the box start), then
    # per-class per-iteration band slabs (h0,h1,w0,w1) with the w-window
    # even-aligned for DVE 2x mode.
    for c in cfg.classes:
        ih, iw = c["ih"], c["iw"]
        ulo = [10**9, 10**9]
        uhi = [-1, -1]
        for b_ in c["cols"]:
            for p in b_[1]:
                offs = []
                for axi, ilen, base in ((1, ih, 1), (2, iw, 2)):
                    ax = p.perm[axi]
                    ext = int(p.ext[ax])
                    if p.srcs:
                        slo = min(s[ax] for s in p.srcs) - int(p.lo[ax])
                        shi = max(s[ax] for s in p.srcs) - int(p.lo[ax])
                    else:
                        slo = shi = ext // 2
                    start = base + (ilen - 1) // 2 - (slo + shi) // 2
                    start = min(max(start, base), base + ilen - ext)
                    offs.append(start)
                    if p.srcs:
                        i01 = 0 if axi == 1 else 1
                        ulo[i01] = min(ulo[i01], start + slo)
                        uhi[i01] = max(uhi[i01], start + shi)
                p.offh, p.offw = offs
        if uhi[0] < 0:  # no sources in this class at all
            ulo, uhi = [1, 2], [ih, iw + 1]
        c["ext_k"] = []
        for k in range(1, N_ITERS + 1):
            h0 = max(1, ulo[0] - k)
            h1 = min(1 + ih, uhi[0] + k + 1)
            w0 = max(2, ulo[1] - k)
            w1 = min(2 + iw, uhi[1] + k + 1)
            w0 -= w0 % 2
            wn = w1 - w0
            wn += wn % 2
            c["ext_k"].append((int(h0), int(h1), int(w0), int(w0 + wn)))

    # Engine schedule. The PE is power-throttled to ~50% duty under
    # sustained load (~1.2 ns/col effective), so DVE 2x (0.53 ns/elem) is
    # the fastest engine per element: the W-axis 3-tap always runs as 2 DVE
    # shift-adds; per (iteration, class) the H-axis 3-tap runs as n_t in
    # {2, 1, 0} further DVE ops (2 = full H on DVE + 1 matmul; 1 = H pair
    # on DVE + 2 matmuls; 0 = H folded into 3 h-shifted matmuls), greedily
    # balancing cumulative DVE vs PE busy-time. The psum->u multiply hops
    # through a chunk-pipelined ScalarE copy (idle engine) for bands >=
    # HOP_MIN so the DVE multiply runs from SBUF at 2x.
    dve = pe = 0.0
    for it in range(N_ITERS):
        for c in cfg.classes:
            j = c["j"]
            h0, h1, w0, w1 = c["ext_k"][it]
            rows, wn = h1 - h0, w1 - w0
            A = rows * wn * j
            nch, rpc = _chunks(rows, wn)
            ach = rpc * wn
            hop = A >= HOP_MIN
            mcost = (nch * (170 + 0.53 * ach) if hop
                     else nch * (192 + 1.25 * ach))
            best = None
            for n in ((2, 1, 0) if c["cls"] == 0 else (2, 1)):
                d = (2 + n) * (170 + 0.53 * A) + mcost * j
                pcost = nch * (3 - n) * (ach * 0.9 + 160) * j
                tot = max(dve + d, pe + pcost)
                if best is None or tot < best[0]:
                    best = (tot, n, d, pcost)
            _, n, d, pcost = best
            dve += d
            pe += pcost
            c.setdefault("sched", []).append(
                dict(n_t=n, hop=hop, nch=nch, rpc=rpc))
    cfg.pieces = pieces
    return cfg


# --------------------------------------------------------------------------
# host-side data packing
# --------------------------------------------------------------------------

def build_inputs(cfg, logits, ea):
    """Build per-core input arrays. Returns in_maps (list of dicts)."""
    bf16 = ml_dtypes.bfloat16
    in_maps = [dict() for _ in range(N_CORES)]
    for ci, c in enumerate(cfg.classes):
        j, bh, bw = c["j"], c["bh"], c["bw"]
        for core in range(N_CORES):
            # prob defaults to 0 outside regions: no heat leaks through box
            # cells beyond a (clipped) region slab, matching the true zero BC.
            in_maps[core][f"pr_{ci}"] = np.zeros((128, j, bh, bw), bf16)
            in_maps[core][f"u0_{ci}"] = np.zeros((128, j, bh, bw), bf16)
            in_maps[core][f"tm_{ci}"] = np.zeros((128, j, 128), bf16)

    inv27 = np.float32(1.0 / 27.0)
    for p in cfg.pieces:
        ci = next(i for i, c in enumerate(cfg.classes) if c["cls"] == p.cls)
        lo, hi, perm = p.lo, p.hi, p.perm
        pe = int(p.ext[perm[0]])
        eh = int(p.ext[perm[1]])
        ew = int(p.ext[perm[2]])
        sl = tuple(slice(int(lo[a]), int(hi[a]) + 1) for a in range(3))
        oh, ow = p.offh, p.offw
        dlt = (logits[p.batch, 1][sl].astype(np.float32)
               - logits[p.batch, 0][sl].astype(np.float32)).transpose(perm)
        prob = 1.0 / (1.0 + np.exp(-dlt))
        in_maps[p.core][f"pr_{ci}"][p.p0:p.p0 + pe, p.col,
                                    oh:oh + eh, ow:ow + ew] = \
            prob.astype(bf16)
        u0 = in_maps[p.core][f"u0_{ci}"]
        for s in p.srcs:
            q = (s[perm[0]] - lo[perm[0]], s[perm[1]] - lo[perm[1]],
                 s[perm[2]] - lo[perm[2]])
            u0[p.p0 + q[0], p.col, oh + q[1], ow + q[2]] = 1.0
        tm = in_maps[p.core][f"tm_{ci}"]
        for i in range(pe):
            for d_ in (-1, 0, 1):
                if 0 <= i + d_ < pe:
                    tm[p.p0 + i, p.col, p.p0 + i + d_] = inv27
    return in_maps


# --------------------------------------------------------------------------
# device kernel
# --------------------------------------------------------------------------

def build_nc(cfg):
    import concourse.bacc as bacc
    import concourse.tile as tile
    from concourse import mybir

    nc = bacc.Bacc("TRN2")
    dram = {}
    for ci, c in enumerate(cfg.classes):
        j, bh, bw = c["j"], c["bh"], c["bw"]
        for nm in ("pr", "u0"):
            dram[f"{nm}_{ci}"] = nc.dram_tensor(
                f"{nm}_{ci}", [128, j, bh, bw], mybir.dt.bfloat16,
                kind="ExternalInput")
        dram[f"tm_{ci}"] = nc.dram_tensor(
            f"tm_{ci}", [128, j, 128], mybir.dt.bfloat16, kind="ExternalInput")
        # full-width row ranges: per-partition contiguous runs so the DMA
        # moves ~2KB packets instead of one tiny packet per row
        for oi in (8, 9, 10):
            h0, h1, w0, w1 = c["ext_k"][oi - 1]
            dram[f"o{oi}_{ci}"] = nc.dram_tensor(
                f"o{oi}_{ci}", [128, j, h1 - h0, bw],
                mybir.dt.bfloat16, kind="ExternalOutput")

    with tile.TileContext(nc) as tc:
        with tc.tile_pool(name="sb", bufs=1) as sb, \
             tc.tile_pool(name="ps", bufs=7, space="PSUM") as pp:
            tiles = []
            for ci, c in enumerate(cfg.classes):
                j, bh, bw = c["j"], c["bh"], c["bw"]
                u = sb.tile([128, j, bh, bw], mybir.dt.bfloat16, tag=f"u{ci}")
                t1 = sb.tile([128, j, bh, bw], mybir.dt.bfloat16,
                             tag=f"t1{ci}")
                t2 = sb.tile([128, j, bh, bw], mybir.dt.bfloat16,
                             tag=f"t2{ci}")
                pr = sb.tile([128, j, bh, bw], mybir.dt.bfloat16,
                             tag=f"pr{ci}")
                tm = sb.tile([128, j, 128], mybir.dt.bfloat16, tag=f"tm{ci}")
                tiles.append((u, t1, t2, pr, tm))
            # inputs spread over three trigger queues (parallel DMA
            # engines); u0_0 gates the first taps, so its transfer is
            # split across two queues to halve the per-packet serial cost
            # tm first (tiny; the first matmuls gate on it), then u0
            for ci, c in enumerate(cfg.classes):
                qa = nc.sync if ci == 0 else nc.gpsimd
                qa.dma_start(out=tiles[ci][4][:], in_=dram[f"tm_{ci}"][:])
            u0t = tiles[0][0]
            nc.sync.dma_start(out=u0t[0:64], in_=dram["u0_0"][0:64])
            nc.gpsimd.dma_start(out=u0t[64:128], in_=dram["u0_0"][64:128])
            for ci, c in enumerate(cfg.classes):
                u, t1, t2, pr, tm = tiles[ci]
                if ci > 0:
                    nc.gpsimd.dma_start(out=u[:], in_=dram[f"u0_{ci}"][:])
                nc.scalar.dma_start(out=pr[:], in_=dram[f"pr_{ci}"][:])

            def emit_taps(ci, it):
                # W-axis 3-tap on DVE (2 shift-adds), written 1 row wider
                # than the band so later H-tap/matmul reads touch only
                # written cells (u's ring is zero from the u0 DMA - no
                # memsets needed). Then the H-axis 3-tap as n_t in {2,1,0}
                # more DVE ops (the rest rides on PE as shifted matmuls).
                c = cfg.classes[ci]
                u, t1, t2, pr, tm = tiles[ci]
                h0, h1, w0, w1 = c["ext_k"][it]
                n_t = c["sched"][it]["n_t"]
                g0, g1 = h0 - 1, h1 + 1
                nc.vector.tensor_add(t1[:, :, g0:g1, w0:w1],
                                     u[:, :, g0:g1, w0 - 1:w1 - 1],
                                     u[:, :, g0:g1, w0 + 1:w1 + 1])
                nc.vector.tensor_add(t1[:, :, g0:g1, w0:w1],
                                     t1[:, :, g0:g1, w0:w1],
                                     u[:, :, g0:g1, w0:w1])
                if n_t >= 1:
                    nc.vector.tensor_add(t2[:, :, h0:h1, w0:w1],
                                         t1[:, :, h0 - 1:h1 - 1, w0:w1],
                                         t1[:, :, h0 + 1:h1 + 1, w0:w1])
                if n_t == 2:
                    nc.vector.tensor_add(t2[:, :, h0:h1, w0:w1],
                                         t2[:, :, h0:h1, w0:w1],
                                         t1[:, :, h0:h1, w0:w1])

            def emit_mms(ci, it):
                # D-axis tridiagonal matmul per chunk; residual H-axis taps
                # ride along as h-shifted accumulating matmuls. Then each
                # psum chunk hops through ScalarE (PSUM->SBUF bf16, idle
                # engine, overwriting t2) for bands >= HOP_MIN.
                c = cfg.classes[ci]
                u, t1, t2, pr, tm = tiles[ci]
                h0, h1, w0, w1 = c["ext_k"][it]
                sch = c["sched"][it]
                n_t, nch, rpc = sch["n_t"], sch["nch"], sch["rpc"]
                if n_t == 2:       # t2 = full H 3-tap sum
                    srcs = [(t2, 0)]
                elif n_t == 1:     # center (t1) first: it depends only on
                    # the W taps, so it overlaps H1 on DVE; t2 accumulates
                    srcs = [(t1, 0), (t2, 0)]
                else:              # full H 3-tap as shifted matmuls
                    srcs = [(t1, 0), (t1, -1), (t1, 1)]
                psums = []
                for jj in range(c["j"]):
                    for ch in range(nch):
                        r0 = h0 + ch * rpc
                        nr = min(rpc, h1 - r0)
                        ps = pp.tile([128, nr, w1 - w0],
                                     mybir.dt.float32, tag="ps")
                        for mi, (src, dh) in enumerate(srcs):
                            nc.tensor.matmul(
                                ps[:],
                                tm[:, jj, :],
                                src[:, jj, r0 + dh:r0 + dh + nr, w0:w1],
                                start=(mi == 0),
                                stop=(mi == len(srcs) - 1))
                        psums.append((jj, r0, nr, ps))
                if sch["hop"]:
                    # chunk 0 skips the hop (its multiply reads PSUM
                    # directly) - it sits first on the critical path and
                    # saving the ScalarE round-trip starts it ~0.6us sooner
                    for ki, (jj, r0, nr, ps) in enumerate(psums):
                        if ki == 0 and len(psums) >= 2:
                            continue
                        nc.scalar.activation(
                            t2[:, jj, r0:r0 + nr, w0:w1], ps[:],
                            mybir.ActivationFunctionType.Copy)
                return psums

            def emit_mults(ci, it, psums):
                # u = psum * prob (1/27 folded into tm), chunk-level so
                # early chunks run while later chunks are still matmuling.
                c = cfg.classes[ci]
                u, t1, t2, pr, tm = tiles[ci]
                h0, h1, w0, w1 = c["ext_k"][it]
                hop = c["sched"][it]["hop"]
                for ki, (jj, r0, nr, ps) in enumerate(psums):
                    direct = (not hop) or (ki == 0 and len(psums) >= 2)
                    src = ps[:] if direct else t2[:, jj, r0:r0 + nr, w0:w1]
                    nc.vector.tensor_mul(u[:, jj, r0:r0 + nr, w0:w1],
                                         src, pr[:, jj, r0:r0 + nr, w0:w1])
                if it >= 7:
                    # gpsimd queue: idle mid-kernel, so the slab transfer
                    # starts immediately and the WAR on u clears sooner
                    nc.gpsimd.dma_start(out=dram[f"o{it + 1}_{ci}"][:],
                                        in_=u[:, :, h0:h1, :])

            # Software-pipelined schedule: class 1's psum->u multiplies are
            # deferred into the next iteration so the in-order DVE queue
            # always has ready work (class 0's taps/multiply) while class
            # 1's matmul+copy chain drains. The Tile scheduler would undo
            # this (its cost model assumes an unthrottled PE and hoists the
            # stalling multiplies), so each phase is pinned with a
            # monotonically increasing bass_wait_until_ts pseudo-time.
            def tw(ns):
                tc.tile_set_cur_wait(ns * 1e-6)

            def dur(ci, it, what):
                c = cfg.classes[ci]
                h0, h1, w0, w1 = c["ext_k"][it]
                A = (h1 - h0) * (w1 - w0) * c["j"]
                sch = c["sched"][it]
                if what == "taps":
                    return (2 + sch["n_t"]) * (190 + 0.55 * A)
                if what == "mult":
                    per = (170 + 0.53 * A / sch["nch"]) if sch["hop"] \
                        else (192 + 1.25 * A / sch["nch"])
                    return sch["nch"] * per
                ach = A / sch["nch"]
                return sch["nch"] * (3 - sch["n_t"]) * (ach * 0.9 + 160)

            if len(cfg.classes) == 1:
                for it in range(N_ITERS):
                    emit_taps(0, it)
                    emit_mults(0, it, emit_mms(0, it))
            else:
                pend = None
                t = 3000.0
                for it in range(N_ITERS - 1):
                    tw(t)
                    emit_taps(0, it)
                    tb = t + dur(0, it, "taps")
                    tw(tb)
                    ps0 = emit_mms(0, it)
                    if pend is not None:
                        tw(tb + 200)
                        emit_mults(1, it - 1, pend)
                        tb += 200 + dur(1, it - 1, "mult")
                    tw(tb + 100)
                    emit_mults(0, it, ps0)
                    tb += 100 + dur(0, it, "mult")
                    tw(tb)
                    emit_taps(1, it)
                    tb += dur(1, it, "taps")
                    tw(tb)
                    pend = emit_mms(1, it)
                    t = tb + 400
                # Final iteration with roles swapped: class 1 (the long
                # matmul+copy chain) is issued first and class 0's short
                # chain drains the tail.
                it = N_ITERS - 1
                tw(t)
                emit_mults(1, it - 1, pend)
                t += dur(1, it - 1, "mult")
                tw(t)
                emit_taps(1, it)
                t += dur(1, it, "taps")
                tw(t)
                ps1 = emit_mms(1, it)
                tw(t + 200)
                emit_taps(0, it)
                t += 200 + dur(0, it, "taps")
                tw(t)
                ps0 = emit_mms(0, it)
                tw(t + 1400)
                emit_mults(1, it, ps1)
                tw(t + 1400 + dur(1, it, "mult"))
                emit_mults(0, it, ps0)
    nc.compile()
    return nc


# --------------------------------------------------------------------------
# host-side finalization
# --------------------------------------------------------------------------

def finalize(cfg, results, eb):
    """results: list of per-core dicts with o8/o9/o10 band slabs."""
    cls_idx = {c["cls"]: i for i, c in enumerate(cfg.classes)}

    # Global maxima of u_8/u_9/u_10 from the slabs; c_10 = mu_10 + eps*mu_9
    # + eps^2*mu_8 + O(eps^3) with eps = SMOOTH = 1e-5 (~1e-15 truncation).
    mus = np.zeros((B, 3), dtype=np.float64)
    for p in cfg.pieces:
        ci = cls_idx[p.cls]
        pe = int(p.ext[p.perm[0]])
        for oi in (8, 9, 10):
            m = results[p.core][f"o{oi}_{ci}"][p.p0:p.p0 + pe, p.col]
            mus[p.batch, oi - 8] = max(mus[p.batch, oi - 8],
                                       float(m.max()))

    per_batch = []
    for b in range(B):
        cscale = 1.0
        for it in range(3):
            if mus[b, it] > 0:
                cscale = mus[b, it] + SMOOTH * cscale
        scores = []
        for e in eb[b]:
            val = 0.0
            for p in cfg.pieces:
                if p.batch != b:
                    continue
                if np.all(p.clo <= e) and np.all(e <= p.chi):
                    ci = cls_idx[p.cls]
                    c = cfg.classes[ci]
                    h0, h1, _, _ = c["ext_k"][N_ITERS - 1]
                    q = (int(e[p.perm[0]] - p.lo[p.perm[0]]),
                         int(e[p.perm[1]] - p.lo[p.perm[1]]),
                         int(e[p.perm[2]] - p.lo[p.perm[2]]))
                    hs = p.offh + q[1] - h0
                    ws = p.offw + q[2]
                    if 0 <= hs < h1 - h0:
                        val = float(results[p.core][f"o10_{ci}"]
                                    [p.p0 + q[0], p.col, hs, ws])
                    break
            scores.append(val / cscale)
        per_batch.append(1.0 - np.float32(np.mean(np.asarray(scores,
                                                             np.float32))))
    return np.array(np.mean(np.asarray(per_batch, np.float32)),
                    dtype=np.float32)


# --------------------------------------------------------------------------
# entry point
# --------------------------------------------------------------------------

def kernel(logits, labels, endpoints_a, endpoints_b):
    global LAST_RESULTS
    logits = np.asarray(logits)
    ea = np.asarray(endpoints_a).astype(np.int64)
    eb = np.asarray(endpoints_b).astype(np.int64)

    cfg = pack(make_pieces(ea))
    in_maps = build_inputs(cfg, logits, ea)
    nc = build_nc(cfg)

    from concourse.bass_utils import run_bass_kernel_spmd
    res = run_bass_kernel_spmd(nc, in_maps, core_ids=list(range(N_CORES)))
    LAST_RESULTS = res
    return finalize(cfg, res.results, eb)


if __name__ == "__main__":
    ins = {k: np.load(f"/tmp/in_{k}.npy")
           for k in ("logits", "labels", "endpoints_a", "endpoints_b")}
    out = kernel(**ins)
    print("kernel loss:", repr(out))



# revision 12
# speedup vs baseline: 1.0329x; 1.0149x over previous
"""CAPE connectivity loss on 8 Trainium2 NeuronCores.

Math (reference): fg_prob = softmax(logits, ch_axis)[:, 1] = sigmoid(l1 - l0);
per batch, heat diffuses from 32 source voxels for 10 iterations of
  h = avg_pool3d_3x3x3(h) * prob;  h /= (max(h) + 1e-5)
then scores = h[endpoints_b], loss = mean over batches of (1 - mean(scores)).

Key structure exploited:
 1. The 3x3x3 box filter dilates support by exactly 1 voxel per iteration, so
    after 10 iterations heat is identically zero outside L-inf radius-10 balls
    around the 32 sources. Compute only on per-cluster regions (bbox+10),
    merged until the expanded regions are pairwise disjoint - then zero-BC
    diffusion per region is exact.
 2. The per-iteration max-normalization commutes with the linear
    pool*prob step: iterate u_{k+1} = pool(u_k)*prob unnormalized, record
    mu_k = max(u_k); then h_k = u_k / c_k with c_k = mu_k + SMOOTH*c_{k-1}.
    Zero-BC values outside a piece's owned claim only *underestimate* the true
    field, and the argmax voxel lies inside some owned claim, so the max over
    all region boxes equals the true global max.
 3. Oversized regions are split along an axis with a +10-voxel halo per
    internal cut; each piece's owned claim stays exact for all 10 iterations.

Device layout: pieces are rotated (largest axis -> partitions), their source
bboxes centered, and packed into columns of [128, j, bh, bw] bf16 SBUF tiles
in two width classes, with zero guard bands around each box (2 cols left so
w-windows stay >= 2). prob = sigmoid(l1-l0) is computed on the host for the
packed boxes only and shipped directly. Per iteration on a band that tracks
the dilated support (even-aligned w-windows for DVE 2x mode), the separable
3x3x3 pool splits engine-adaptively: the D-axis 3-tap is a PE matmul against
a block-tridiagonal 1/27 matrix; the W-axis 3-tap always rides on PE as
w-shifted accumulating matmuls; the H-axis 3-tap runs as 0/1/2 DVE row-shift
adds with the remainder folded into extra h-shifted matmuls (3/6/9 matmuls
per chunk) - chosen per (class, iteration) by a cost model balancing DVE vs
PE load. The *prob multiply runs on DVE, reading PSUM directly for small
bands or via a ScalarE PSUM->SBUF copy (idle engine) for large ones. u is
updated in place; band slabs of u_8/u_9/u_10 are DMAed out and the host
computes the global maxima (scale recurrence c_10 = mu_10 + eps*mu_9 +
eps^2*mu_8 + O(eps^3)) and gathers endpoint_b values. A burst of dummy
matmuls at kernel start warms the PE HAM clock gate during the input DMA.
"""

import ml_dtypes
import numpy as np

B, C, D, H, W = 2, 2, 128, 256, 256
N_PAIRS = 32
N_ITERS = 10
SMOOTH = 1e-5
R = N_ITERS  # diffusion reach in voxels
VOL = (D, H, W)

AXIS_CAP = 34   # max free-axis extent of a piece (split with halo beyond)
P_CAP = 128     # max partition-axis extent
A_FREE = 21     # class-A pieces have both free extents <= A_FREE
N_CORES = 8
N_WARM = 64     # PE warmup matmuls (HAM un-throttle needs ~3.4us busy)
HOP_MIN = 256   # min band area for the ScalarE PSUM->SBUF hop

# Results of the last device run (for test harness introspection).
LAST_RESULTS = None


# --------------------------------------------------------------------------
# planning: clusters -> regions -> pieces
# --------------------------------------------------------------------------

class Piece:
    __slots__ = ("lo", "hi", "clo", "chi", "srcs", "batch",
                 "perm", "cls", "core", "col", "p0", "offh", "offw")

    def __init__(self, lo, hi, clo, chi, srcs, batch):
        self.lo = np.asarray(lo); self.hi = np.asarray(hi)
        self.clo = np.asarray(clo); self.chi = np.asarray(chi)
        self.srcs = srcs
        self.batch = batch

    @property
    def ext(self):
        return self.hi - self.lo + 1


def _merge_clusters(pts):
    """Merge clusters until expanded regions (bbox+R) are pairwise disjoint."""
    clusters = [[i] for i in range(len(pts))]

    def bbox(c):
        p = pts[c]
        return p.min(0), p.max(0)

    changed = True
    while changed:
        changed = False
        out = []
        while clusters:
            c = clusters.pop()
            lo_c, hi_c = bbox(c)
            for k, o in enumerate(clusters):
                lo_o, hi_o = bbox(o)
                if np.all(lo_c - hi_o <= 2 * R) and np.all(lo_o - hi_c <= 2 * R):
                    clusters[k] = o + c
                    changed = True
                    break
            else:
                out.append(c)
        clusters = out
    return clusters


def _split(piece):
    """Split a piece until partition extent <= P_CAP and free extents <=
    AXIS_CAP. Claims are halved; each internal cut adds R halo."""
    out, done = [piece], []
    while out:
        p = out.pop()
        ext = p.ext
        order = np.argsort(-ext, kind="stable")
        ax = None
        if ext[order[0]] > P_CAP:
            ax = order[0]
        elif ext[order[1]] > AXIS_CAP:
            ax = order[1]
        elif ext[order[2]] > AXIS_CAP:
            ax = order[2]
        if ax is None:
            done.append(p)
            continue
        mid = (p.clo[ax] + p.chi[ax]) // 2
        for a, b in ((p.clo[ax], mid), (mid + 1, p.chi[ax])):
            nclo, nchi = p.clo.copy(), p.chi.copy()
            nclo[ax], nchi[ax] = a, b
            nlo, nhi = p.lo.copy(), p.hi.copy()
            nlo[ax] = max(a - R, p.lo[ax])
            nhi[ax] = min(b + R, p.hi[ax])
            srcs = [s for s in p.srcs if nlo[ax] <= s[ax] <= nhi[ax]]
            out.append(Piece(nlo, nhi, nclo, nchi, srcs, p.batch))
    return done


def make_pieces(ea):
    pieces = []
    for b in range(ea.shape[0]):
        pts = np.unique(ea[b], axis=0)
        for cl in _merge_clusters(pts):
            p = pts[cl]
            lo = np.maximum(p.min(0) - R, 0)
            hi = np.minimum(p.max(0) + R, np.asarray(VOL) - 1)
            base = Piece(lo, hi, lo, hi, [tuple(x) for x in p], b)
            pieces.extend(_split(base))
    return pieces


# --------------------------------------------------------------------------
# packing: pieces -> (class, core, col, partition offset) + schedule
# --------------------------------------------------------------------------

class Cfg:
    pass


def _chunks(rows, wn):
    nch = max(1, -(-(rows * wn) // 512))
    rpc = -(-rows // nch)
    while rpc * wn > 512:
        nch += 1
        rpc = -(-rows // nch)
    return nch, rpc


def pack(pieces):
    """Assign each piece a rotation + (class, core, col, p0); build the
    per-(class, iteration) band slabs, chunking, and engine schedule."""
    for p in pieces:
        ext = p.ext
        p.perm = tuple(int(i) for i in np.argsort(-ext, kind="stable"))

    def free_ext(p):
        e = p.ext
        return e[p.perm[1]], e[p.perm[2]]

    cls_of = {}
    for p in pieces:
        fh, fw = free_ext(p)
        p.cls = 0 if (fh <= A_FREE and fw <= A_FREE) else 1
        cls_of.setdefault(p.cls, []).append(p)

    cfg = Cfg()
    cfg.classes = []
    for cls in (0, 1):
        plist = cls_of.get(cls, [])
        if not plist:
            continue
        ih = max(int(free_ext(p)[0]) for p in plist)
        iw = max(int(free_ext(p)[1]) for p in plist)
        bh = ih + 2
        bw = iw + 4 + ((iw + 4) % 2)  # 2 guard cols left, >=2 right, even
        # first-fit decreasing bin pack by partition extent
        plist.sort(key=lambda p: -int(p.ext[p.perm[0]]))
        bins = []  # (used, [pieces])
        for p in plist:
            pe = int(p.ext[p.perm[0]])
            for b_ in bins:
                if b_[0] + pe <= 128:
                    p.p0 = b_[0]
                    b_[0] += pe
                    b_[1].append(p)
                    break
            else:
                p.p0 = 0
                bins.append([pe, [p]])
        cfg.classes.append(dict(cls=cls, ih=ih, iw=iw, bh=bh, bw=bw,
                                cols=bins))

    for c in cfg.classes:
        for i, b_ in enumerate(c["cols"]):
            for p in b_[1]:
                p.core = i % N_CORES
                p.col = i // N_CORES
        c["j"] = (len(c["cols"]) + N_CORES - 1) // N_CORES

    # Center each piece's source bbox in its box interior (rows start at 1,
    # cols at 2 so shifted matmul reads never cross the box start), then
    # per-class per-iteration band slabs (h0,h1,w0,w1) with the w-window
    # even-aligned for DVE 2x mode.
    for c in cfg.classes:
        ih, iw = c["ih"], c["iw"]
        ulo = [10**9, 10**9]
        uhi = [-1, -1]
        for b_ in c["cols"]:
            for p in b_[1]:
                offs = []
                for axi, ilen, base in ((1, ih, 1), (2, iw, 2)):
                    ax = p.perm[axi]
                    ext = int(p.ext[ax])
                    if p.srcs:
                        slo = min(s[ax] for s in p.srcs) - int(p.lo[ax])
                        shi = max(s[ax] for s in p.srcs) - int(p.lo[ax])
                    else:
                        slo = shi = ext // 2
                    start = base + (ilen - 1) // 2 - (slo + shi) // 2
                    start = min(max(start, base), base + ilen - ext)
                    offs.append(start)
                    if p.srcs:
                        i01 = 0 if axi == 1 else 1
                        ulo[i01] = min(ulo[i01], start + slo)
                        uhi[i01] = max(uhi[i01], start + shi)
                p.offh, p.offw = offs
        if uhi[0] < 0:  # no sources in this class at all
            ulo, uhi = [1, 2], [ih, iw + 1]
        c["ext_k"] = []
        for k in range(1, N_ITERS + 1):
            h0 = max(1, ulo[0] - k)
            h1 = min(1 + ih, uhi[0] + k + 1)
            w0 = max(2, ulo[1] - k)
            w1 = min(2 + iw, uhi[1] + k + 1)
            w0 -= w0 % 2
            wn = w1 - w0
            wn += wn % 2
            c["ext_k"].append((int(h0), int(h1), int(w0), int(w0 + wn)))

    # Engine schedule. The PE is power-throttled to ~50% duty under
    # sustained load (~1.2 ns/col effective), so DVE 2x (0.53 ns/elem) is
    # the fastest engine per element: the W-axis 3-tap always runs as 2 DVE
    # shift-adds; per (iteration, class) the H-axis 3-tap runs as n_t in
    # {2, 1, 0} further DVE ops (2 = full H on DVE + 1 matmul; 1 = H pair
    # on DVE + 2 matmuls; 0 = H folded into 3 h-shifted matmuls), greedily
    # balancing cumulative DVE vs PE busy-time. The psum->u multiply hops
    # through a chunk-pipelined ScalarE copy (idle engine) for bands >=
    # HOP_MIN so the DVE multiply runs from SBUF at 2x.
    dve = pe = 0.0
    for it in range(N_ITERS):
        for c in cfg.classes:
            j = c["j"]
            h0, h1, w0, w1 = c["ext_k"][it]
            rows, wn = h1 - h0, w1 - w0
            A = rows * wn * j
            nch, rpc = _chunks(rows, wn)
            ach = rpc * wn
            hop = A >= HOP_MIN
            mcost = (nch * (170 + 0.53 * ach) if hop
                     else nch * (192 + 1.25 * ach))
            best = None
            for n in ((2, 1, 0) if c["cls"] == 0 else (2, 1)):
                d = (2 + n) * (170 + 0.53 * A) + mcost * j
                pcost = nch * (3 - n) * (ach * 0.9 + 160) * j
                tot = max(dve + d, pe + pcost)
                if best is None or tot < best[0]:
                    best = (tot, n, d, pcost)
            _, n, d, pcost = best
            dve += d
            pe += pcost
            c.setdefault("sched", []).append(
                dict(n_t=n, hop=hop, nch=nch, rpc=rpc))
    cfg.pieces = pieces
    return cfg


# --------------------------------------------------------------------------
# host-side data packing
# --------------------------------------------------------------------------

def build_inputs(cfg, logits, ea):
    """Build per-core input arrays. Returns in_maps (list of dicts)."""
    bf16 = ml_dtypes.bfloat16
    in_maps = [dict() for _ in range(N_CORES)]
    for ci, c in enumerate(cfg.classes):
        j, bh, bw = c["j"], c["bh"], c["bw"]
        for core in range(N_CORES):
            # prob defaults to 0 outside regions: no heat leaks through box
            # cells beyond a (clipped) region slab, matching the true zero BC.
            in_maps[core][f"pr_{ci}"] = np.zeros((128, j, bh, bw), bf16)
            in_maps[core][f"u0_{ci}"] = np.zeros((128, j, bh, bw), bf16)
            in_maps[core][f"tm_{ci}"] = np.zeros((128, j, 128), bf16)

    inv27 = np.float32(1.0 / 27.0)
    for p in cfg.pieces:
        ci = next(i for i, c in enumerate(cfg.classes) if c["cls"] == p.cls)
        lo, hi, perm = p.lo, p.hi, p.perm
        pe = int(p.ext[perm[0]])
        eh = int(p.ext[perm[1]])
        ew = int(p.ext[perm[2]])
        sl = tuple(slice(int(lo[a]), int(hi[a]) + 1) for a in range(3))
        oh, ow = p.offh, p.offw
        dlt = (logits[p.batch, 1][sl].astype(np.float32)
               - logits[p.batch, 0][sl].astype(np.float32)).transpose(perm)
        prob = 1.0 / (1.0 + np.exp(-dlt))
        in_maps[p.core][f"pr_{ci}"][p.p0:p.p0 + pe, p.col,
                                    oh:oh + eh, ow:ow + ew] = \
            prob.astype(bf16)
        u0 = in_maps[p.core][f"u0_{ci}"]
        for s in p.srcs:
            q = (s[perm[0]] - lo[perm[0]], s[perm[1]] - lo[perm[1]],
                 s[perm[2]] - lo[perm[2]])
            u0[p.p0 + q[0], p.col, oh + q[1], ow + q[2]] = 1.0
        tm = in_maps[p.core][f"tm_{ci}"]
        for i in range(pe):
            for d_ in (-1, 0, 1):
                if 0 <= i + d_ < pe:
                    tm[p.p0 + i, p.col, p.p0 + i + d_] = inv27
    return in_maps


# --------------------------------------------------------------------------
# device kernel
# --------------------------------------------------------------------------

def build_nc(cfg):
    import concourse.bacc as bacc
    import concourse.tile as tile
    from concourse import mybir

    nc = bacc.Bacc("TRN2")
    dram = {}
    for ci, c in enumerate(cfg.classes):
        j, bh, bw = c["j"], c["bh"], c["bw"]
        for nm in ("pr", "u0"):
            dram[f"{nm}_{ci}"] = nc.dram_tensor(
                f"{nm}_{ci}", [128, j, bh, bw], mybir.dt.bfloat16,
                kind="ExternalInput")
        dram[f"tm_{ci}"] = nc.dram_tensor(
            f"tm_{ci}", [128, j, 128], mybir.dt.bfloat16, kind="ExternalInput")
        # full-width row ranges: per-partition contiguous runs so the DMA
        # moves ~2KB packets instead of one tiny packet per row
        for oi in (8, 9, 10):
            h0, h1, w0, w1 = c["ext_k"][oi - 1]
            dram[f"o{oi}_{ci}"] = nc.dram_tensor(
                f"o{oi}_{ci}", [128, j, h1 - h0, bw],
                mybir.dt.bfloat16, kind="ExternalOutput")

    with tile.TileContext(nc) as tc:
        with tc.tile_pool(name="sb", bufs=1) as sb, \
             tc.tile_pool(name="ps", bufs=7, space="PSUM") as pp:
            tiles = []
            for ci, c in enumerate(cfg.classes):
                j, bh, bw = c["j"], c["bh"], c["bw"]
                u = sb.tile([128, j, bh, bw], mybir.dt.bfloat16, tag=f"u{ci}")
                t1 = sb.tile([128, j, bh, bw], mybir.dt.bfloat16,
                             tag=f"t1{ci}")
                t2 = sb.tile([128, j, bh, bw], mybir.dt.bfloat16,
                             tag=f"t2{ci}")
                pr = sb.tile([128, j, bh, bw], mybir.dt.bfloat16,
                             tag=f"pr{ci}")
                tm = sb.tile([128, j, 128], mybir.dt.bfloat16, tag=f"tm{ci}")
                tiles.append((u, t1, t2, pr, tm))
            # inputs spread over three trigger queues (parallel DMA
            # engines); u0_0 gates the first taps, so its transfer is
            # split across two queues to halve the per-packet serial cost
            # tm rides the scalar queue head: tiny, lands ~9.8us for the
            # first matmuls, without delaying u0_0 (sync queue head) which
            # gates the very first taps. pr follows tm; it's not needed
            # until the first multiply (~13us).
            for ci, c in enumerate(cfg.classes):
                nc.scalar.dma_start(out=tiles[ci][4][:],
                                    in_=dram[f"tm_{ci}"][:])
            u0t = tiles[0][0]
            nc.sync.dma_start(out=u0t[0:64], in_=dram["u0_0"][0:64])
            nc.gpsimd.dma_start(out=u0t[64:128], in_=dram["u0_0"][64:128])
            for ci, c in enumerate(cfg.classes):
                u, t1, t2, pr, tm = tiles[ci]
                if ci > 0:
                    nc.gpsimd.dma_start(out=u[:], in_=dram[f"u0_{ci}"][:])
                nc.scalar.dma_start(out=pr[:], in_=dram[f"pr_{ci}"][:])

            def emit_taps(ci, it):
                # W-axis 3-tap on DVE (2 shift-adds), written 1 row wider
                # than the band so later H-tap/matmul reads touch only
                # written cells (u's ring is zero from the u0 DMA - no
                # memsets needed). Then the H-axis 3-tap as n_t in {2,1,0}
                # more DVE ops (the rest rides on PE as shifted matmuls).
                c = cfg.classes[ci]
                u, t1, t2, pr, tm = tiles[ci]
                h0, h1, w0, w1 = c["ext_k"][it]
                n_t = c["sched"][it]["n_t"]
                g0, g1 = h0 - 1, h1 + 1
                nc.vector.tensor_add(t1[:, :, g0:g1, w0:w1],
                                     u[:, :, g0:g1, w0 - 1:w1 - 1],
                                     u[:, :, g0:g1, w0 + 1:w1 + 1])
                nc.vector.tensor_add(t1[:, :, g0:g1, w0:w1],
                                     t1[:, :, g0:g1, w0:w1],
                                     u[:, :, g0:g1, w0:w1])
                if n_t >= 1:
                    nc.vector.tensor_add(t2[:, :, h0:h1, w0:w1],
                                         t1[:, :, h0 - 1:h1 - 1, w0:w1],
                                         t1[:, :, h0 + 1:h1 + 1, w0:w1])
                if n_t == 2:
                    nc.vector.tensor_add(t2[:, :, h0:h1, w0:w1],
                                         t2[:, :, h0:h1, w0:w1],
                                         t1[:, :, h0:h1, w0:w1])

            def emit_mms(ci, it):
                # D-axis tridiagonal matmul per chunk; residual H-axis taps
                # ride along as h-shifted accumulating matmuls. Then each
                # psum chunk hops through ScalarE (PSUM->SBUF bf16, idle
                # engine, overwriting t2) for bands >= HOP_MIN.
                c = cfg.classes[ci]
                u, t1, t2, pr, tm = tiles[ci]
                h0, h1, w0, w1 = c["ext_k"][it]
                sch = c["sched"][it]
                n_t, nch, rpc = sch["n_t"], sch["nch"], sch["rpc"]
                if n_t == 2:       # t2 = full H 3-tap sum
                    srcs = [(t2, 0)]
                elif n_t == 1:     # center (t1) first: it depends only on
                    # the W taps, so it overlaps H1 on DVE; t2 accumulates
                    srcs = [(t1, 0), (t2, 0)]
                else:              # full H 3-tap as shifted matmuls
                    srcs = [(t1, 0), (t1, -1), (t1, 1)]
                psums = []
                for jj in range(c["j"]):
                    for ch in range(nch):
                        r0 = h0 + ch * rpc
                        nr = min(rpc, h1 - r0)
                        ps = pp.tile([128, nr, w1 - w0],
                                     mybir.dt.float32, tag="ps")
                        for mi, (src, dh) in enumerate(srcs):
                            nc.tensor.matmul(
                                ps[:],
                                tm[:, jj, :],
                                src[:, jj, r0 + dh:r0 + dh + nr, w0:w1],
                                start=(mi == 0),
                                stop=(mi == len(srcs) - 1))
                        psums.append((jj, r0, nr, ps))
                if sch["hop"]:
                    # chunk 0 skips the hop (its multiply reads PSUM
                    # directly) - it sits first on the critical path and
                    # saving the ScalarE round-trip starts it ~0.6us sooner
                    for ki, (jj, r0, nr, ps) in enumerate(psums):
                        if ki == 0 and len(psums) >= 2:
                            continue
                        nc.scalar.activation(
                            t2[:, jj, r0:r0 + nr, w0:w1], ps[:],
                            mybir.ActivationFunctionType.Copy)
                return psums

            def emit_mults(ci, it, psums):
                # u = psum * prob (1/27 folded into tm), chunk-level so
                # early chunks run while later chunks are still matmuling.
                c = cfg.classes[ci]
                u, t1, t2, pr, tm = tiles[ci]
                h0, h1, w0, w1 = c["ext_k"][it]
                hop = c["sched"][it]["hop"]
                for ki, (jj, r0, nr, ps) in enumerate(psums):
                    direct = (not hop) or (ki == 0 and len(psums) >= 2)
                    src = ps[:] if direct else t2[:, jj, r0:r0 + nr, w0:w1]
                    nc.vector.tensor_mul(u[:, jj, r0:r0 + nr, w0:w1],
                                         src, pr[:, jj, r0:r0 + nr, w0:w1])
                if it >= 7:
                    # gpsimd queue: idle mid-kernel, so the slab transfer
                    # starts immediately and the WAR on u clears sooner
                    nc.gpsimd.dma_start(out=dram[f"o{it + 1}_{ci}"][:],
                                        in_=u[:, :, h0:h1, :])

            # Software-pipelined schedule: class 1's psum->u multiplies are
            # deferred into the next iteration so the in-order DVE queue
            # always has ready work (class 0's taps/multiply) while class
            # 1's matmul+copy chain drains. The Tile scheduler would undo
            # this (its cost model assumes an unthrottled PE and hoists the
            # stalling multiplies), so each phase is pinned with a
            # monotonically increasing bass_wait_until_ts pseudo-time.
            def tw(ns):
                tc.tile_set_cur_wait(ns * 1e-6)

            def dur(ci, it, what):
                c = cfg.classes[ci]
                h0, h1, w0, w1 = c["ext_k"][it]
                A = (h1 - h0) * (w1 - w0) * c["j"]
                sch = c["sched"][it]
                if what == "taps":
                    return (2 + sch["n_t"]) * (190 + 0.55 * A)
                if what == "mult":
                    per = (170 + 0.53 * A / sch["nch"]) if sch["hop"] \
                        else (192 + 1.25 * A / sch["nch"])
                    return sch["nch"] * per
                ach = A / sch["nch"]
                return sch["nch"] * (3 - sch["n_t"]) * (ach * 0.9 + 160)

            if len(cfg.classes) == 1:
                for it in range(N_ITERS):
                    emit_taps(0, it)
                    emit_mults(0, it, emit_mms(0, it))
            else:
                pend = None
                t = 3000.0
                for it in range(N_ITERS - 1):
                    tw(t)
                    emit_taps(0, it)
                    tb = t + dur(0, it, "taps")
                    tw(tb)
                    ps0 = emit_mms(0, it)
                    if pend is not None:
                        tw(tb + 200)
                        emit_mults(1, it - 1, pend)
                        tb += 200 + dur(1, it - 1, "mult")
                    tw(tb + 100)
                    emit_mults(0, it, ps0)
                    tb += 100 + dur(0, it, "mult")
                    tw(tb)
                    emit_taps(1, it)
                    tb += dur(1, it, "taps")
                    tw(tb)
                    pend = emit_mms(1, it)
                    t = tb + 400
                # Final iteration with roles swapped: class 1 (the long
                # matmul+copy chain) is issued first and class 0's short
                # chain drains the tail.
                it = N_ITERS - 1
                tw(t)
                emit_mults(1, it - 1, pend)
                t += dur(1, it - 1, "mult")
                tw(t)
                emit_taps(1, it)
                t += dur(1, it, "taps")
                tw(t)
                ps1 = emit_mms(1, it)
                tw(t + 200)
                emit_taps(0, it)
                t += 200 + dur(0, it, "taps")
                tw(t)
                ps0 = emit_mms(0, it)
                tw(t + 1400)
                emit_mults(1, it, ps1)
                tw(t + 1400 + dur(1, it, "mult"))
                emit_mults(0, it, ps0)
    nc.compile()
    return nc


# --------------------------------------------------------------------------
# host-side finalization
# --------------------------------------------------------------------------

def finalize(cfg, results, eb):
    """results: list of per-core dicts with o8/o9/o10 band slabs."""
    cls_idx = {c["cls"]: i for i, c in enumerate(cfg.classes)}

    # Global maxima of u_8/u_9/u_10 from the slabs; c_10 = mu_10 + eps*mu_9
    # + eps^2*mu_8 + O(eps^3) with eps = SMOOTH = 1e-5 (~1e-15 truncation).
    mus = np.zeros((B, 3), dtype=np.float64)
    for p in cfg.pieces:
        ci = cls_idx[p.cls]
        pe = int(p.ext[p.perm[0]])
        for oi in (8, 9, 10):
            m = results[p.core][f"o{oi}_{ci}"][p.p0:p.p0 + pe, p.col]
            mus[p.batch, oi - 8] = max(mus[p.batch, oi - 8],
                                       float(m.max()))

    per_batch = []
    for b in range(B):
        cscale = 1.0
        for it in range(3):
            if mus[b, it] > 0:
                cscale = mus[b, it] + SMOOTH * cscale
        scores = []
        for e in eb[b]:
            val = 0.0
            for p in cfg.pieces:
                if p.batch != b:
                    continue
                if np.all(p.clo <= e) and np.all(e <= p.chi):
                    ci = cls_idx[p.cls]
                    c = cfg.classes[ci]
                    h0, h1, _, _ = c["ext_k"][N_ITERS - 1]
                    q = (int(e[p.perm[0]] - p.lo[p.perm[0]]),
                         int(e[p.perm[1]] - p.lo[p.perm[1]]),
                         int(e[p.perm[2]] - p.lo[p.perm[2]]))
                    hs = p.offh + q[1] - h0
                    ws = p.offw + q[2]
                    if 0 <= hs < h1 - h0:
                        val = float(results[p.core][f"o10_{ci}"]
                                    [p.p0 + q[0], p.col, hs, ws])
                    break
            scores.append(val / cscale)
        per_batch.append(1.0 - np.float32(np.mean(np.asarray(scores,
                                                             np.float32))))
    return np.array(np.mean(np.asarray(per_batch, np.float32)),
                    dtype=np.float32)


# --------------------------------------------------------------------------
# entry point
# --------------------------------------------------------------------------

def kernel(logits, labels, endpoints_a, endpoints_b):
    global LAST_RESULTS
    logits = np.asarray(logits)
    ea = np.asarray(endpoints_a).astype(np.int64)
    eb = np.asarray(endpoints_b).astype(np.int64)

    cfg = pack(make_pieces(ea))
    in_maps = build_inputs(cfg, logits, ea)
    nc = build_nc(cfg)

    from concourse.bass_utils import run_bass_kernel_spmd
    res = run_bass_kernel_spmd(nc, in_maps, core_ids=list(range(N_CORES)))
    LAST_RESULTS = res
    return finalize(cfg, res.results, eb)


if __name__ == "__main__":
    ins = {k: np.load(f"/tmp/in_{k}.npy")
           for k in ("logits", "labels", "endpoints_a", "endpoints_b")}
    out = kernel(**ins)
    print("kernel loss:", repr(out))

